# revision 24
# baseline (speedup 1.0000x reference)
"""CompressAttn Trainium2 Bass kernel (v3: head-mixed tiles, host norm).

Problem: compressed-block attention.
  B=2, N=4096, QH=32, KH=2, D=VD=128, KSZ=32, STRIDE=16, M=255 blocks.
  kc[b,m,h,:] = sum_i w_k[i] * (k[b,16m+i,h,:] + pe_k[i,:])   (same for v)
  out = softmax(q @ kc^T * D^-0.5, causal-banded mask) @ vc, zero for n < 31.

Sharding: 8 cores = (batch b in {0,1}) x (query-head quarter hq in {0..3}).
Each core handles 8 query heads sharing one KV head; K/V compression done
once per core.  No collectives; host gathers + normalizes.

Device pipeline per 64-query tile t (q columns are head-mixed: 64 queries x
8 heads = 512 moving columns per matmul, so all matmuls stream 512 cols):
  1. QK:   sT[m_c, 512] = kcT_c^T @ q_t        (1-2 chunk matmuls -> psum)
  2. exp:  eT = exp(sm * sT) on ScalarE, one activation spanning both
           psum banks when 2 chunks.
  3. mask: ONE full-height [128,512] multiply on GpSimd; variant v=t%32
           marks staircase rows 4v-1..4v+2 (else 1.0), +1 extra op at t=32
           for row 127 of chunk 0.
  4. PV:   oT[vd, 512] += vc_c^T(natural, stationary) @ eT_c  (psum)
  5. den:  den[32 dup rows, 512] += ones32^T @ eT_c, 4 tiles per psum
           bank at partition 32*(t%4) (explicit tile_position for 96).
  6. Drains: PV psum -> bf16 SBUF on DVE; den psum -> bf16 SBUF on ACT;
     output DMAs batched 4 tiles.  Softmax division on HOST
     (out_unnorm / den, zero where den==0).
Pipeline: lookahead-2 (qk(t) | exp/mask(t-1) | pv+den(t-2)) keeps the PE
from stalling on the exp->mask chain.
"""

import ml_dtypes
import numpy as np

import concourse.bacc as bacc
import concourse.mybir as mybir
import concourse.tile as tile
from concourse.bass_utils import run_bass_kernel_spmd

# Problem geometry (hardcoded).
B, N, QH, KH, D, VD = 2, 4096, 32, 2, 128, 128
KSZ, STRIDE = 32, 16
M = (N - KSZ) // STRIDE + 1          # 255 compressed blocks
HPC = QH // 4                         # 8 query heads per core
NT = 64                               # 64-query tiles per core
SM = float(D) ** -0.5
NMV = 33                              # mask variants (t%32 full + A127)

F32 = mybir.dt.float32
BF16 = mybir.dt.bfloat16
EXP = mybir.ActivationFunctionType.Exp


def build_program():
    nc = bacc.Bacc("TRN2", target_bir_lowering=False, debug=False)

    qT_d = nc.dram_tensor("qT", [128, NT * 512], BF16, kind="ExternalInput")
    k_d = nc.dram_tensor("kk", [128, 32 * 128], BF16, kind="ExternalInput")
    v_d = nc.dram_tensor("vv", [128, 32 * 128], BF16, kind="ExternalInput")
    w01k_d = nc.dram_tensor("w01k", [128, 16], BF16, kind="ExternalInput")
    w01v_d = nc.dram_tensor("w01v", [128, 16], BF16, kind="ExternalInput")
    bk_d = nc.dram_tensor("biask", [128, 1], F32, kind="ExternalInput")
    bv_d = nc.dram_tensor("biasv", [128, 1], F32, kind="ExternalInput")
    id_d = nc.dram_tensor("ident", [128, 128], F32, kind="ExternalInput")
    io_d = nc.dram_tensor("iota", [128, 512], BF16, kind="ExternalInput")
    th_d = nc.dram_tensor("thr", [128, NMV], F32, kind="ExternalInput")
    oT_d = nc.dram_tensor("oT", [128, NT * 512], BF16, kind="ExternalOutput")
    den_d = nc.dram_tensor("den", [16, 128, 512], BF16, kind="ExternalOutput")

    with tile.TileContext(nc) as tc:
        with tc.tile_pool(name="consts", bufs=1) as cp:
            w01k = cp.tile([128, 16], BF16)
            w01v = cp.tile([128, 16], BF16)
            biask = cp.tile([128, 1], F32)
            biasv = cp.tile([128, 1], F32)
            ident = cp.tile([128, 128], F32)
            iota = cp.tile([128, 512], BF16)
            thr = cp.tile([128, NMV], F32)
            ones32 = cp.tile([128, 32], BF16)
            ktile = cp.tile([128, 32 * 128], BF16)
            vtile = cp.tile([128, 32 * 128], BF16)
            kcT = cp.tile([128, 256], BF16)       # [d, m] (col 255 = 0 pad)
            vcT = cp.tile([128, 256], F32)        # [d, m] staging
            vca0 = cp.tile([128, 128], BF16)      # vc natural, m 0:128
            vca1 = cp.tile([128, 128], BF16)      # vc natural, m 128:255(+pad)
            qsb = cp.tile([128, NT * 512], BF16)  # [d, (t h j)]

            nc.sync.dma_start(ktile[:, 0:2048], k_d.ap()[:, 0:2048])
            nc.scalar.dma_start(vtile[:, 0:2048], v_d.ap()[:, 0:2048])
            nc.sync.dma_start(w01k[:, :], w01k_d.ap())
            nc.scalar.dma_start(w01v[:, :], w01v_d.ap())
            nc.sync.dma_start(ktile[:, 2048:4096], k_d.ap()[:, 2048:4096])
            nc.scalar.dma_start(vtile[:, 2048:4096], v_d.ap()[:, 2048:4096])
            nc.sync.dma_start(biask[:, :], bk_d.ap())
            nc.scalar.dma_start(biasv[:, :], bv_d.ap())
            nc.scalar.dma_start(ident[:, :], id_d.ap())
            nc.scalar.dma_start(iota[:, :], io_d.ap())
            nc.scalar.dma_start(thr[:, :], th_d.ap())
            nc.vector.memset(ones32[:, :], 1.0)
            for s in range(8):
                eng = nc.sync if s % 2 == 0 else nc.scalar
                eng.dma_start(
                    qsb[:, 4096 * s : 4096 * (s + 1)],
                    qT_d.ap()[:, 4096 * s : 4096 * (s + 1)],
                )

            # ---- attention ----
            def tile_geom(t):
                ctot = 4 * t + 3            # visible m count (= min(.,255))
                c0 = min(ctot, 128)
                c1 = ctot - 128
                return c0, c1

            state = {}

            def emit_qk(t, sps, qbufs=None):
                c0, c1 = tile_geom(t)
                nblk = 2 if c1 > 0 else 1
                sp = sps.tile([128, 512 * nblk], F32, tag=f"sp{nblk}",
                              bufs=qbufs, name="sp")
                nc.tensor.matmul(
                    sp[:, 0:512], kcT[:, 0:128],
                    qsb[:, 512 * t : 512 * (t + 1)],
                    start=True, stop=True,
                )
                if c1 > 0:
                    nc.tensor.matmul(
                        sp[:, 512:1024], kcT[:, 128:256],
                        qsb[:, 512 * t : 512 * (t + 1)],
                        start=True, stop=True,
                    )
                state[t] = sp

            def emit_exp_mask(t, ep):
                c0, c1 = tile_geom(t)
                nblk = 2 if c1 > 0 else 1
                sp = state.pop(t)
                eT = ep.tile([128, 512 * nblk], BF16, tag=f"eT{nblk}")
                nc.scalar.activation(eT[:, :], sp[:, :], EXP, scale=SM)
                # staircase mask via affine select (no constants): query
                # j of head h sees row p iff j - 16p + 64v - 31 >= 0, where
                # v = t (chunk0, t<=32 w/ A127 variant 32) or t-32 (chunk1).
                # This also zeroes all rows above the staircase.
                def sel(eng, v, coff):
                    if eng is nc.gpsimd:
                        e3 = eT[:, coff : coff + 512].rearrange(
                            "p (h j) -> p h j", h=8)
                        eng.affine_select(
                            out=e3, in_=e3,
                            pattern=[[0, 8], [1, 64]],
                            compare_op=mybir.AluOpType.is_ge,
                            fill=0.0,
                            base=64 * v - 31,
                            channel_multiplier=-16,
                        )
                    else:
                        es = eT[:, coff : coff + 512]
                        eng.scalar_tensor_tensor(
                            out=es, in0=iota[:, :], in1=es,
                            scalar=thr[:, v : v + 1],
                            op0=mybir.AluOpType.is_ge,
                            op1=mybir.AluOpType.mult,
                        )

                eng = nc.gpsimd if (t < 32 and t % 2 == 0) else nc.vector
                if t < 32:
                    sel(eng, t, 0)
                elif t == 32:
                    sel(nc.vector, 0, 512)
                    sel(nc.gpsimd, 32, 0)
                else:
                    sel(eng, t - 32, 512)
                state[t] = eT

            # ---- compression (attention pools open first so early qk
            # tiles can use them) ----
            ep = tc.alloc_tile_pool(name="ep", bufs=9)
            with tc.tile_pool(name="ppsum", bufs=1, space="PSUM") as pp:
                early_sps = pp
                pkT = pp.tile([128, 512], F32)   # [d, (T a)]
                pvT = pp.tile([128, 512], F32)
                tpA = pp.tile([128, 128], F32)
                tpB = pp.tile([128, 128], F32)
                for c in range(32):
                    nc.tensor.matmul(
                        pkT[:, 16 * c : 16 * c + 16],
                        ktile[:, 128 * c : 128 * (c + 1)],
                        w01k[:, :],
                        start=True, stop=True,
                    )
                # kcT[d,m] = P0[m] + P1[m+1] + bias_k[d]  (runs on DVE while
                # the PE streams v compression)
                pk3 = pkT[:, :].rearrange("p (t a) -> p t a", a=2)
                pv3 = pvT[:, :].rearrange("p (t a) -> p t a", a=2)
                nc.vector.memset(kcT[:, M : M + 1], 0.0)
                nc.vector.tensor_scalar_add(kcT[:, 0:M], pk3[:, 0:M, 0], biask[:, 0:1])
                nc.vector.tensor_add(kcT[:, 0:M], kcT[:, 0:M], pk3[:, 1 : M + 1, 1])
                for c in range(32):
                    nc.tensor.matmul(
                        pvT[:, 16 * c : 16 * c + 16],
                        vtile[:, 128 * c : 128 * (c + 1)],
                        w01v[:, :],
                        start=True, stop=True,
                    )
                    if c % 8 == 7:
                        t_early = c // 8          # qk(0..3) warm the pipeline
                        emit_qk(t_early, early_sps, qbufs=2)
                        emit_exp_mask(t_early, ep)
                nc.vector.tensor_scalar_add(vcT[:, 0:M], pv3[:, 0:M, 0], biasv[:, 0:1])
                nc.vector.tensor_add(vcT[:, 0:M], vcT[:, 0:M], pv3[:, 1 : M + 1, 1])
                nc.vector.memset(vcT[:, M : M + 1], 0.0)
                nc.tensor.transpose(tpA[:, :], vcT[:, 0:128], ident[:, :])
                nc.tensor.transpose(tpB[:, :], vcT[:, 128:256], ident[:, :])
                nc.vector.tensor_copy(vca0[:, :], tpA[:, :])
                nc.vector.tensor_copy(vca1[:, :], tpB[:, :])

            # ---- attention emitters (defined below, used in comp too) ----
            def emit_pv(t, pvs, obp):
                c0, c1 = tile_geom(t)
                eT = state[t]
                po = pvs.tile([128, 512], F32, tag="po")
                nc.tensor.matmul(
                    po[:, :], vca0[:, :], eT[:, 0:512],
                    start=True, stop=True, skip_group_check=True,
                )
                if c1 > 0:
                    nc.tensor.matmul(
                        po[:, :], vca1[:, :], eT[:, 512:1024],
                        start=False, stop=True, skip_group_check=True,
                    )
                r = t % 4
                # drain PV psum -> bf16 staging (batch 4 tiles per DMA)
                if r == 0:
                    state["ob"] = obp.tile([128, 2048], BF16, tag="ob", name="ob")
                ob = state["ob"]
                nc.vector.tensor_copy(ob[:, 512 * r : 512 * (r + 1)], po[:, :])
                if r == 3:
                    nc.sync.dma_start(
                        oT_d.ap()[:, 2048 * (t // 4) : 2048 * (t // 4 + 1)],
                        ob[:, :],
                    )

            def emit_den_burst(group, dnp, dbp):
                # 4 tiles at 4 PE column-quadrant positions run concurrently;
                # chunk0 of all tiles first, then accumulating chunk1s.
                dn = dnp.tile([128, 512], F32, tag="dn", name="dn")
                for t in group:
                    r = t % 4
                    nc.tensor.matmul(
                        dn[32 * r : 32 * r + 32, :], ones32[:, :],
                        state[t][:, 0:512],
                        start=True, stop=True, skip_group_check=True,
                        tile_position=(0, 32 * r),
                    )
                for t in group:
                    c0, c1 = tile_geom(t)
                    if c1 > 0:
                        r = t % 4
                        nc.tensor.matmul(
                            dn[32 * r : 32 * r + 32, :], ones32[:, :],
                            state[t][:, 512:1024],
                            start=False, stop=True, skip_group_check=True,
                            tile_position=(0, 32 * r),
                        )
                for t in group:
                    state.pop(t)
                t3 = group[-1]
                db = dbp.tile([128, 512], BF16, tag="db", name="db")
                if t3 < 32:
                    nc.scalar.copy(db[:, :], dn[:, :])
                else:
                    nc.vector.tensor_copy(db[:, :], dn[:, :])
                nc.sync.dma_start(den_d.ap()[t3 // 4], db[:, :])

            pv_q, den_q = [0, 1, 2, 3], [0, 1, 2, 3]

            def drain(pvs, dns, obp, dbp, la):
                # pv per tile; dens burst 4-at-a-time (they run concurrently
                # on distinct PE column-quadrant tile positions)
                while len(pv_q) > la:
                    t0 = pv_q.pop(0)
                    emit_pv(t0, pvs, obp)
                    if t0 % 4 == 3:
                        group = [den_q.pop(0) for _ in range(4)]
                        emit_den_burst(group, dns, dbp)

            with (
                tc.tile_pool(name="obp", bufs=2) as obp,
                tc.tile_pool(name="dbp", bufs=2) as dbp,
                tc.tile_pool(name="pvs", bufs=2, space="PSUM") as pvs,
                tc.tile_pool(name="dns", bufs=2, space="PSUM") as dns,
            ):
                with tc.tile_pool(name="spsA", bufs=4, space="PSUM") as spsA:
                    for t in range(4, 32):
                        emit_qk(t, spsA)
                        emit_exp_mask(t, ep)
                        pv_q.append(t)
                        den_q.append(t)
                        drain(pvs, dns, obp, dbp, 4)
                with tc.tile_pool(name="spsB", bufs=2, space="PSUM") as spsB:
                    for t in range(32, 64):
                        emit_qk(t, spsB)
                        emit_exp_mask(t, ep)
                        pv_q.append(t)
                        den_q.append(t)
                        drain(pvs, dns, obp, dbp, 2)
                    while pv_q:
                        t0 = pv_q.pop(0)
                        emit_pv(t0, pvs, obp)
                        if t0 % 4 == 3:
                            group = [den_q.pop(0) for _ in range(4)]
                            emit_den_burst(group, dns, dbp)
            ep.release()
    nc.compile()
    return nc


def make_consts(w_k, pe_k, w_v, pe_v):
    """Host-side constant tensors fed to every core."""
    f = np.float32
    w01k = np.zeros((128, 16), f)
    w01v = np.zeros((128, 16), f)
    for r in range(128):
        j = r // 16
        s = r % 16
        for a in range(2):
            w01k[r, 2 * j + a] = w_k[16 * a + s]
            w01v[r, 2 * j + a] = w_v[16 * a + s]
    biask = (w_k[:, None] * pe_k).sum(0).astype(f)[:, None]  # [128,1]
    biasv = (w_v[:, None] * pe_v).sum(0).astype(f)[:, None]
    ident = np.eye(128, dtype=f)
    iota = np.tile(np.arange(64, dtype=f), (128, 8)).reshape(128, 512)
    # visible iff j >= 16p - 64v + 31
    pp, vv = np.meshgrid(np.arange(128), np.arange(NMV), indexing="ij")
    thr = (16 * pp - 64 * vv + 31).astype(f)
    bf = ml_dtypes.bfloat16
    return {
        "w01k": w01k.astype(bf),
        "w01v": w01v.astype(bf),
        "biask": biask,
        "biasv": biasv,
        "ident": ident,
        "iota": iota.astype(bf),
        "thr": thr,
    }


def make_in_map(q, k, v, consts, core):
    b, hq = core // 4, core % 4
    g = hq // 2
    bf = ml_dtypes.bfloat16
    # qT layout [d, t, h, j]: q[b, 64t+j, 8hq+h, d]
    qc = q[b, :, 8 * hq : 8 * (hq + 1), :]            # [N, 8, 128]
    qT = np.ascontiguousarray(
        qc.reshape(64, 64, 8, 128).transpose(3, 0, 2, 1)
    ).reshape(128, -1).astype(bf)
    return {
        "qT": qT,
        "kk": np.ascontiguousarray(
            k[b, :, g, :].reshape(32, 128, 128).transpose(1, 0, 2)
        ).reshape(128, -1).astype(bf),
        "vv": np.ascontiguousarray(
            v[b, :, g, :].reshape(32, 128, 128).transpose(1, 0, 2)
        ).reshape(128, -1).astype(bf),
        **consts,
    }


_CACHE = {}


def _compiled():
    if "nc" not in _CACHE:
        _CACHE["nc"] = build_program()
    return _CACHE["nc"]


def kernel(q, k, v, w_k, pe_k, w_v, pe_v, _trace=False, _trace_kwargs=None):
    q = np.asarray(q, np.float32)
    k = np.asarray(k, np.float32)
    v = np.asarray(v, np.float32)
    consts = make_consts(
        np.asarray(w_k, np.float32), np.asarray(pe_k, np.float32),
        np.asarray(w_v, np.float32), np.asarray(pe_v, np.float32),
    )
    nc = _compiled()
    in_maps = [make_in_map(q, k, v, consts, c) for c in range(8)]
    kw = {}
    if _trace:
        kw = {"trace": True, **(_trace_kwargs or {})}
    res = run_bass_kernel_spmd(nc, in_maps, core_ids=list(range(8)), **kw)
    out = np.empty((B, N, QH, VD), np.float32)
    for c in range(8):
        b, hq = c // 4, c % 4
        oT = res.results[c]["oT"].astype(np.float32)      # [128, 64*512]
        den = res.results[c]["den"].astype(np.float32)    # [16, 128, 512]
        num = oT.reshape(128, 64, 8, 64).transpose(1, 3, 2, 0)  # [t, j, h, d]
        dsel = den[:, (0, 32, 64, 96), :].reshape(64, 8, 64)    # [t, h, j]
        dsel = dsel.transpose(0, 2, 1)[:, :, :, None]           # [t, j, h, 1]
        o = np.where(dsel > 0, num / np.maximum(dsel, 1e-30), 0.0)
        out[b, :, 8 * hq : 8 * (hq + 1), :] = o.reshape(N, HPC, VD)
    _CACHE["last_result"] = res
    return out


# revision 25
# speedup vs baseline: 1.0481x; 1.0481x over previous
"""CompressAttn Trainium2 Bass kernel (v3: head-mixed tiles, host norm).

Problem: compressed-block attention.
  B=2, N=4096, QH=32, KH=2, D=VD=128, KSZ=32, STRIDE=16, M=255 blocks.
  kc[b,m,h,:] = sum_i w_k[i] * (k[b,16m+i,h,:] + pe_k[i,:])   (same for v)
  out = softmax(q @ kc^T * D^-0.5, causal-banded mask) @ vc, zero for n < 31.

Sharding: 8 cores = (batch b in {0,1}) x (query-head quarter hq in {0..3}).
Each core handles 8 query heads sharing one KV head; K/V compression done
once per core.  No collectives; host gathers + normalizes.

Device pipeline per 64-query tile t (q columns are head-mixed: 64 queries x
8 heads = 512 moving columns per matmul, so all matmuls stream 512 cols):
  1. QK:   sT[m_c, 512] = kcT_c^T @ q_t        (1-2 chunk matmuls -> psum)
  2. exp:  eT = exp(sm * sT) on ScalarE, one activation spanning both
           psum banks when 2 chunks.
  3. mask: ONE full-height [128,512] multiply on GpSimd; variant v=t%32
           marks staircase rows 4v-1..4v+2 (else 1.0), +1 extra op at t=32
           for row 127 of chunk 0.
  4. PV:   oT[vd, 512] += vc_c^T(natural, stationary) @ eT_c  (psum)
  5. den:  den[32 dup rows, 512] += ones32^T @ eT_c, 4 tiles per psum
           bank at partition 32*(t%4) (explicit tile_position for 96).
  6. Drains: PV psum -> bf16 SBUF on DVE; den psum -> bf16 SBUF on ACT;
     output DMAs batched 4 tiles.  Softmax division on HOST
     (out_unnorm / den, zero where den==0).
Pipeline: lookahead-2 (qk(t) | exp/mask(t-1) | pv+den(t-2)) keeps the PE
from stalling on the exp->mask chain.
"""

import ml_dtypes
import numpy as np

import concourse.bacc as bacc
import concourse.mybir as mybir
import concourse.tile as tile
from concourse.bass_utils import run_bass_kernel_spmd

# Problem geometry (hardcoded).
B, N, QH, KH, D, VD = 2, 4096, 32, 2, 128, 128
KSZ, STRIDE = 32, 16
M = (N - KSZ) // STRIDE + 1          # 255 compressed blocks
HPC = QH // 4                         # 8 query heads per core
NT = 64                               # 64-query tiles per core
SM = float(D) ** -0.5
NMV = 33                              # mask variants (t%32 full + A127)

F32 = mybir.dt.float32
BF16 = mybir.dt.bfloat16
EXP = mybir.ActivationFunctionType.Exp


def build_program():
    nc = bacc.Bacc("TRN2", target_bir_lowering=False, debug=False)

    qT_d = nc.dram_tensor("qT", [128, NT * 512], BF16, kind="ExternalInput")
    k_d = nc.dram_tensor("kk", [128, 32 * 128], BF16, kind="ExternalInput")
    v_d = nc.dram_tensor("vv", [128, 32 * 128], BF16, kind="ExternalInput")
    w01k_d = nc.dram_tensor("w01k", [128, 16], BF16, kind="ExternalInput")
    w01v_d = nc.dram_tensor("w01v", [128, 16], BF16, kind="ExternalInput")
    bk_d = nc.dram_tensor("biask", [128, 1], F32, kind="ExternalInput")
    bv_d = nc.dram_tensor("biasv", [128, 1], F32, kind="ExternalInput")
    id_d = nc.dram_tensor("ident", [128, 128], F32, kind="ExternalInput")
    mk_d = nc.dram_tensor("maskc", [128, NMV * 512], BF16, kind="ExternalInput")
    oT_d = nc.dram_tensor("oT", [128, NT * 512], BF16, kind="ExternalOutput")
    den_d = nc.dram_tensor("den", [16, 128, 512], BF16, kind="ExternalOutput")

    with tile.TileContext(nc) as tc:
        with tc.tile_pool(name="consts", bufs=1) as cp:
            w01k = cp.tile([128, 16], BF16)
            w01v = cp.tile([128, 16], BF16)
            biask = cp.tile([128, 1], F32)
            biasv = cp.tile([128, 1], F32)
            ident = cp.tile([128, 128], F32)
            maskc = cp.tile([128, NMV * 512], BF16)
            ones32 = cp.tile([128, 32], BF16)
            ktile = cp.tile([128, 32 * 128], BF16)
            vtile = cp.tile([128, 32 * 128], BF16)
            kcT = cp.tile([128, 256], BF16)       # [d, m] (col 255 = 0 pad)
            vcT = cp.tile([128, 256], F32)        # [d, m] staging
            vca0 = cp.tile([128, 128], BF16)      # vc natural, m 0:128
            vca1 = cp.tile([128, 128], BF16)      # vc natural, m 128:255(+pad)
            qsb = cp.tile([128, NT * 512], BF16)  # [d, (t h j)]

            nc.sync.dma_start(ktile[:, :], k_d.ap())
            nc.sync.dma_start(vtile[:, :], v_d.ap())
            nc.sync.dma_start(w01k[:, :], w01k_d.ap())
            nc.sync.dma_start(w01v[:, :], w01v_d.ap())
            nc.sync.dma_start(biask[:, :], bk_d.ap())
            nc.sync.dma_start(biasv[:, :], bv_d.ap())
            nc.sync.dma_start(qsb[:, 0:4096], qT_d.ap()[:, 0:4096])
            nc.sync.dma_start(ident[:, :], id_d.ap())
            nc.sync.dma_start(maskc[:, 0:4096], mk_d.ap()[:, 0:4096])
            nc.vector.memset(ones32[:, :], 1.0)
            for s in range(1, 8):
                nc.sync.dma_start(
                    qsb[:, 4096 * s : 4096 * (s + 1)],
                    qT_d.ap()[:, 4096 * s : 4096 * (s + 1)],
                )
                if s == 1:
                    nc.sync.dma_start(maskc[:, 4096:8192],
                                      mk_d.ap()[:, 4096:8192])
                if s == 2:
                    nc.sync.dma_start(maskc[:, 8192 : NMV * 512],
                                      mk_d.ap()[:, 8192 : NMV * 512])

            # ---- attention ----
            def tile_geom(t):
                ctot = 4 * t + 3            # visible m count (= min(.,255))
                c0 = min(ctot, 128)
                c1 = ctot - 128
                return c0, c1

            state = {}

            def emit_qk(t, sps, qbufs=None):
                c0, c1 = tile_geom(t)
                nblk = 2 if c1 > 0 else 1
                sp = sps.tile([128, 512 * nblk], F32, tag=f"sp{nblk}",
                              bufs=qbufs, name="sp")
                nc.tensor.matmul(
                    sp[:, 0:512], kcT[:, 0:128],
                    qsb[:, 512 * t : 512 * (t + 1)],
                    start=True, stop=True,
                )
                if c1 > 0:
                    nc.tensor.matmul(
                        sp[:, 512:1024], kcT[:, 128:256],
                        qsb[:, 512 * t : 512 * (t + 1)],
                        start=True, stop=True,
                    )
                state[t] = sp

            def emit_exp_mask(t, ep):
                c0, c1 = tile_geom(t)
                nblk = 2 if c1 > 0 else 1
                sp = state.pop(t)
                eT = ep.tile([128, 512 * nblk], BF16, tag=f"eT{nblk}")
                nc.scalar.activation(eT[:, :], sp[:, :], EXP, scale=SM)
                # staircase mask via affine select (no constants): query
                # j of head h sees row p iff j - 16p + 64v - 31 >= 0, where
                # v = t (chunk0, t<=32 w/ A127 variant 32) or t-32 (chunk1).
                # This also zeroes all rows above the staircase.
                def sel(eng, v, coff):
                    eng.tensor_mul(
                        eT[:, coff : coff + 512], eT[:, coff : coff + 512],
                        maskc[:, 512 * v : 512 * (v + 1)],
                    )

                eng = nc.gpsimd if (t < 32 and t % 2 == 0) else nc.vector
                if t < 32:
                    sel(eng, t, 0)
                elif t == 32:
                    sel(nc.vector, 0, 512)
                    sel(nc.gpsimd, 32, 0)
                else:
                    sel(eng, t - 32, 512)
                state[t] = eT

            # ---- compression (attention pools open first so early qk
            # tiles can use them) ----
            ep = tc.alloc_tile_pool(name="ep", bufs=9)
            with tc.tile_pool(name="ppsum", bufs=1, space="PSUM") as pp:
                early_sps = pp
                pkT = pp.tile([128, 512], F32)   # [d, (T a)]
                pvT = pp.tile([128, 512], F32)
                tpA = pp.tile([128, 128], F32)
                tpB = pp.tile([128, 128], F32)
                for c in range(32):
                    nc.tensor.matmul(
                        pkT[:, 16 * c : 16 * c + 16],
                        ktile[:, 128 * c : 128 * (c + 1)],
                        w01k[:, :],
                        start=True, stop=True,
                    )
                # kcT[d,m] = P0[m] + P1[m+1] + bias_k[d]  (runs on DVE while
                # the PE streams v compression)
                pk3 = pkT[:, :].rearrange("p (t a) -> p t a", a=2)
                pv3 = pvT[:, :].rearrange("p (t a) -> p t a", a=2)
                nc.vector.memset(kcT[:, M : M + 1], 0.0)
                nc.vector.tensor_scalar_add(kcT[:, 0:M], pk3[:, 0:M, 0], biask[:, 0:1])
                nc.vector.tensor_add(kcT[:, 0:M], kcT[:, 0:M], pk3[:, 1 : M + 1, 1])
                for c in range(32):
                    nc.tensor.matmul(
                        pvT[:, 16 * c : 16 * c + 16],
                        vtile[:, 128 * c : 128 * (c + 1)],
                        w01v[:, :],
                        start=True, stop=True,
                    )
                    if c % 8 == 7:
                        t_early = c // 8          # qk(0..3) warm the pipeline
                        emit_qk(t_early, early_sps, qbufs=2)
                        emit_exp_mask(t_early, ep)
                nc.vector.tensor_scalar_add(vcT[:, 0:M], pv3[:, 0:M, 0], biasv[:, 0:1])
                nc.vector.tensor_add(vcT[:, 0:M], vcT[:, 0:M], pv3[:, 1 : M + 1, 1])
                nc.vector.memset(vcT[:, M : M + 1], 0.0)
                nc.tensor.transpose(tpA[:, :], vcT[:, 0:128], ident[:, :])
                nc.tensor.transpose(tpB[:, :], vcT[:, 128:256], ident[:, :])
                nc.vector.tensor_copy(vca0[:, :], tpA[:, :])
                nc.vector.tensor_copy(vca1[:, :], tpB[:, :])

            # ---- attention emitters (defined below, used in comp too) ----
            def emit_pv(t, pvs, obp):
                c0, c1 = tile_geom(t)
                eT = state[t]
                po = pvs.tile([128, 512], F32, tag="po")
                nc.tensor.matmul(
                    po[:, :], vca0[:, :], eT[:, 0:512],
                    start=True, stop=True, skip_group_check=True,
                )
                if c1 > 0:
                    nc.tensor.matmul(
                        po[:, :], vca1[:, :], eT[:, 512:1024],
                        start=False, stop=True, skip_group_check=True,
                    )
                r = t % 4
                # drain PV psum -> bf16 staging (batch 4 tiles per DMA)
                if r == 0:
                    state["ob"] = obp.tile([128, 2048], BF16, tag="ob", name="ob")
                ob = state["ob"]
                nc.vector.tensor_copy(ob[:, 512 * r : 512 * (r + 1)], po[:, :])
                if r == 3:
                    nc.sync.dma_start(
                        oT_d.ap()[:, 2048 * (t // 4) : 2048 * (t // 4 + 1)],
                        ob[:, :],
                    )

            def emit_den_burst(group, dnp, dbp):
                # 4 tiles at 4 PE column-quadrant positions run concurrently;
                # chunk0 of all tiles first, then accumulating chunk1s.
                dn = dnp.tile([128, 512], F32, tag="dn", name="dn")
                for t in group:
                    r = t % 4
                    nc.tensor.matmul(
                        dn[32 * r : 32 * r + 32, :], ones32[:, :],
                        state[t][:, 0:512],
                        start=True, stop=True, skip_group_check=True,
                        tile_position=(0, 32 * r),
                    )
                for t in group:
                    c0, c1 = tile_geom(t)
                    if c1 > 0:
                        r = t % 4
                        nc.tensor.matmul(
                            dn[32 * r : 32 * r + 32, :], ones32[:, :],
                            state[t][:, 512:1024],
                            start=False, stop=True, skip_group_check=True,
                            tile_position=(0, 32 * r),
                        )
                for t in group:
                    state.pop(t)
                t3 = group[-1]
                db = dbp.tile([128, 512], BF16, tag="db", name="db")
                if t3 < 32:
                    nc.scalar.copy(db[:, :], dn[:, :])
                else:
                    nc.vector.tensor_copy(db[:, :], dn[:, :])
                nc.sync.dma_start(den_d.ap()[t3 // 4], db[:, :])

            pv_q, den_q = [0, 1, 2, 3], [0, 1, 2, 3]

            def drain(pvs, dns, obp, dbp, la):
                # pv per tile; dens burst 4-at-a-time (they run concurrently
                # on distinct PE column-quadrant tile positions)
                while len(pv_q) > la:
                    t0 = pv_q.pop(0)
                    emit_pv(t0, pvs, obp)
                    if t0 % 4 == 3:
                        group = [den_q.pop(0) for _ in range(4)]
                        emit_den_burst(group, dns, dbp)

            with (
                tc.tile_pool(name="obp", bufs=2) as obp,
                tc.tile_pool(name="dbp", bufs=2) as dbp,
                tc.tile_pool(name="pvs", bufs=2, space="PSUM") as pvs,
                tc.tile_pool(name="dns", bufs=2, space="PSUM") as dns,
            ):
                with tc.tile_pool(name="spsA", bufs=4, space="PSUM") as spsA:
                    for t in range(4, 32):
                        emit_qk(t, spsA)
                        emit_exp_mask(t, ep)
                        pv_q.append(t)
                        den_q.append(t)
                        drain(pvs, dns, obp, dbp, 4)
                with tc.tile_pool(name="spsB", bufs=2, space="PSUM") as spsB:
                    for t in range(32, 64):
                        emit_qk(t, spsB)
                        emit_exp_mask(t, ep)
                        pv_q.append(t)
                        den_q.append(t)
                        drain(pvs, dns, obp, dbp, 2)
                    while pv_q:
                        t0 = pv_q.pop(0)
                        emit_pv(t0, pvs, obp)
                        if t0 % 4 == 3:
                            group = [den_q.pop(0) for _ in range(4)]
                            emit_den_burst(group, dns, dbp)
            ep.release()
    nc.compile()
    return nc


def make_consts(w_k, pe_k, w_v, pe_v):
    """Host-side constant tensors fed to every core."""
    f = np.float32
    w01k = np.zeros((128, 16), f)
    w01v = np.zeros((128, 16), f)
    for r in range(128):
        j = r // 16
        s = r % 16
        for a in range(2):
            w01k[r, 2 * j + a] = w_k[16 * a + s]
            w01v[r, 2 * j + a] = w_v[16 * a + s]
    biask = (w_k[:, None] * pe_k).sum(0).astype(f)[:, None]  # [128,1]
    biasv = (w_v[:, None] * pe_v).sum(0).astype(f)[:, None]
    ident = np.eye(128, dtype=f)
    # full-height staircase masks; staircase row with in-stair index delta is
    # visible iff j >= 15 + 16*delta; rows above the staircase -> 0.
    maskc = np.ones((128, NMV, 512), f)
    jj = np.arange(64)

    def stair(delta):
        return np.tile((jj >= 15 + 16 * delta).astype(f), 8)

    for v in range(32):
        for delta in range(4):
            p = 4 * v - 1 + delta
            if 0 <= p < 128:
                maskc[p, v] = stair(delta)
        maskc[4 * v + 3 :, v] = 0.0
    maskc[127, 32] = stair(0)
    maskc = maskc.reshape(128, NMV * 512)
    bf = ml_dtypes.bfloat16
    return {
        "w01k": w01k.astype(bf),
        "w01v": w01v.astype(bf),
        "biask": biask,
        "biasv": biasv,
        "ident": ident,
        "maskc": maskc.astype(bf),
    }


def make_in_map(q, k, v, consts, core):
    b, hq = core // 4, core % 4
    g = hq // 2
    bf = ml_dtypes.bfloat16
    # qT layout [d, t, h, j]: q[b, 64t+j, 8hq+h, d]
    qc = q[b, :, 8 * hq : 8 * (hq + 1), :]            # [N, 8, 128]
    qT = np.ascontiguousarray(
        qc.reshape(64, 64, 8, 128).transpose(3, 0, 2, 1)
    ).reshape(128, -1).astype(bf)
    return {
        "qT": qT,
        "kk": np.ascontiguousarray(
            k[b, :, g, :].reshape(32, 128, 128).transpose(1, 0, 2)
        ).reshape(128, -1).astype(bf),
        "vv": np.ascontiguousarray(
            v[b, :, g, :].reshape(32, 128, 128).transpose(1, 0, 2)
        ).reshape(128, -1).astype(bf),
        **consts,
    }


_CACHE = {}


def _compiled():
    if "nc" not in _CACHE:
        _CACHE["nc"] = build_program()
    return _CACHE["nc"]


def kernel(q, k, v, w_k, pe_k, w_v, pe_v, _trace=False, _trace_kwargs=None):
    q = np.asarray(q, np.float32)
    k = np.asarray(k, np.float32)
    v = np.asarray(v, np.float32)
    consts = make_consts(
        np.asarray(w_k, np.float32), np.asarray(pe_k, np.float32),
        np.asarray(w_v, np.float32), np.asarray(pe_v, np.float32),
    )
    nc = _compiled()
    in_maps = [make_in_map(q, k, v, consts, c) for c in range(8)]
    kw = {}
    if _trace:
        kw = {"trace": True, **(_trace_kwargs or {})}
    res = run_bass_kernel_spmd(nc, in_maps, core_ids=list(range(8)), **kw)
    out = np.empty((B, N, QH, VD), np.float32)
    for c in range(8):
        b, hq = c // 4, c % 4
        oT = res.results[c]["oT"].astype(np.float32)      # [128, 64*512]
        den = res.results[c]["den"].astype(np.float32)    # [16, 128, 512]
        num = oT.reshape(128, 64, 8, 64).transpose(1, 3, 2, 0)  # [t, j, h, d]
        dsel = den[:, (0, 32, 64, 96), :].reshape(64, 8, 64)    # [t, h, j]
        dsel = dsel.transpose(0, 2, 1)[:, :, :, None]           # [t, j, h, 1]
        o = np.where(dsel > 0, num / np.maximum(dsel, 1e-30), 0.0)
        out[b, :, 8 * hq : 8 * (hq + 1), :] = o.reshape(N, HPC, VD)
    _CACHE["last_result"] = res
    return out
